# revision 25
# baseline (speedup 1.0000x reference)
"""Gaussian blur 101x101 (separable) on 4096x4096 fp32, 8 NeuronCores.

Strategy: the 2D kernel W = outer(gv, gh) is rank-1, so the blur is two 1D
101-tap convs. Rows are sharded 512/core; each core gets a host-prepared
padded fp16 strip (50-row halo, zero-padded edges) so the on-device program
is uniform across cores with no collectives.

Each 1D conv maps onto the PE array as banded matmuls with 128-row
contraction windows and 128-wide output chunks. Because TAPS=101 < 128,
each 128-output chunk needs exactly 2 contraction windows (256 cycles per
128x128 output tile — the K=128 floor). Adjacent chunks share windows, so
per window ONE "straddling" N=256 matmul writes both neighbouring chunks
at once (lower half accumulates via band G1, upper half starts via G0):
PSUM's per-element has_written bit turns first-touch into overwrite and
second-touch into accumulate, with start=True on the first matmul of the
bank marking the whole 2KB bank pending-zero.

  pass1: tmT[j', 512a + i] = sum_r x[r, j'] gv[r - i]   (5 MMs per window a)
  pass2: y[i, j] = sum_j' tmT[j', i] gh[j' - j]         (10 MMs per (g2, q))

Everything lives in fp16 (x strip, band tiles, tm intermediate, y output)
with fp32 PSUM accumulation: halves DMA traffic vs fp32 and enables fast
weight load; rel err ~5e-4 vs the 2e-2 gate.

The input strip is relaid out chunk-major on the host (all 5 row-windows
of a column chunk contiguous) so each chunk is a single contiguous 2D DMA;
output rounds go out as one 3D-AP DMA covering all four 128-row blocks.
"""

import os
import time
from contextlib import ExitStack

import numpy as np

import concourse.bass as bass  # noqa: F401  (AP types come via tile/bacc)
import concourse.mybir as mybir
import concourse.tile as tile
from concourse import bacc, bass_utils

H = 4096
W = 4096
TAPS = 101
PAD = 50
N_CORES = 8
RPC = H // N_CORES          # 512 output rows per core
NW1 = 5                     # input row windows of 128 per core
XP_ROWS = 128 * NW1         # 640 = 512 + 100 halo + 28 slack (zeros)
NA = 33                     # tmT column windows of 128
XP_COLS = 128 * NA          # 4224 = 50 + 4096 + 78 (cols incl zero pads)
CCUTS = [0, 512, 1024, 1536, 2560, 3584, XP_COLS]
DT = mybir.dt.float32

_compiled = {}


class _FastExitTC(tile.TileContext):
    """TileContext whose exit skips the per-semaphore clear storm.

    The stock exit emits dma_reset + sem_clear for every allocated semaphore
    plus a second all-engine barrier — pure tail on a NEFF that is loaded,
    executed once, and unloaded. The drain + one barrier (which gate
    output-DMA completion) are kept.
    """

    def _drain_and_barrier(self, tick_clock, wait_clock):
        from concourse.vector_clock import ScopedClock

        drain_inst = self.nc.sync.drain()
        wait_clock.add_sem_waits(
            drain_inst.ins, ScopedClock({None: tick_clock.global_clock})
        )
        popped = self.nc._tile_sem_poison_stack.pop()
        assert popped is self._sem_poison


def _stoff(g):
    # column offset of (g, q)'s 512-wide slot within its staging tile
    if g >= 6:
        return lambda q: 512 * q
    return lambda q, gl=g % 2: 1024 * q + 512 * gl


def _chunk_of(a):
    c0 = 128 * a
    for ci in range(len(CCUTS) - 1):
        if CCUTS[ci] <= c0 < CCUTS[ci + 1]:
            return ci
    raise AssertionError(a)


def _build_nc(mm_dtype):
    nc = bacc.Bacc(
        "TRN2",
        target_bir_lowering=False,
        debug=False,
        enable_asserts=False,
        num_devices=N_CORES,
    )
    # chunk-major relaid strip: chunk ci cols [5*cs, 5*ce) hold the 5 row
    # windows of strip cols [cs, ce) side by side
    xp = nc.dram_tensor(
        "xp", [128, NW1 * XP_COLS], mm_dtype, kind="ExternalInput"
    ).ap()
    bandst = nc.dram_tensor(
        "bandst", [128, 512], mm_dtype, kind="ExternalInput"
    ).ap()
    y = nc.dram_tensor("y", [RPC, W], mm_dtype, kind="ExternalOutput").ap()

    with _FastExitTC(nc) as tc, ExitStack() as ctx:
        xw_pool = ctx.enter_context(tc.tile_pool(name="xw", bufs=1))
        band_pool = ctx.enter_context(tc.tile_pool(name="bands", bufs=1))
        tm_pool = ctx.enter_context(tc.tile_pool(name="tm", bufs=1))
        p1_pool = ctx.enter_context(tc.tile_pool(name="p1", bufs=4, space="PSUM"))
        p2_pool = ctx.enter_context(tc.tile_pool(name="p2", bufs=3, space="PSUM"))
        st_pool = ctx.enter_context(tc.tile_pool(name="st", bufs=2))

        xw = xw_pool.tile([128, NW1 * XP_COLS], mm_dtype, tag="xw", name="xw")
        tm = tm_pool.tile([128, 512 * NA], mm_dtype, tag="tm", name="tm")

        def lhsT1(w, a):
            ci = _chunk_of(a)
            cs, ce = CCUTS[ci], CCUTS[ci + 1]
            off = 5 * cs + w * (ce - cs) + (128 * a - cs)
            return xw[:, off : off + 128]

        # PE warmup: fp16 matmuls on a DVE-memset scratch tile need no DMA,
        # so they run while the first input chunks are still in flight.
        wt = band_pool.tile([128, 512], mm_dtype, tag="wt", name="wt")
        nc.vector.memset(wt[:], 0.0)
        wps = p2_pool.tile([128, 512], DT, name="wps", tag="ps2")
        for _ in range(8):
            nc.tensor.matmul(
                wps[:], lhsT=wt[:, 0:128], rhs=wt[:], start=True, stop=True
            )

        # two HWDGE rings: inputs ride scalar's (its issues run before any
        # cast work lands there), outputs ride sync's — so output bursts
        # never queue behind input chunks within one in-order ring.
        def dma(dst, src):
            nc.sync.dma_start(dst, src)

        bt = band_pool.tile([128, 512], mm_dtype, tag="bt", name="bt")
        nc.scalar.dma_start(bt[:], bandst[:])
        for ci in range(len(CCUTS) - 1):
            cs, ce = 5 * CCUTS[ci], 5 * CCUTS[ci + 1]
            nc.scalar.dma_start(xw[:, cs:ce], xp[:, cs:ce])

        # band tile column layout: [Gv1 | Gv0 | Gh1 | Gh0]
        GV1, GV0, GH1, GH0 = 0, 128, 256, 384

        cast_k = 0

        def cast(dst, src):
            nonlocal cast_k
            eng = [nc.vector.tensor_copy, nc.scalar.copy][cast_k % 2]
            cast_k += 1
            eng(dst, src)

        def pass1_a(a):
            """Window a -> tm[:, 512a:+512]."""
            ps1 = p1_pool.tile([128, 512], DT, tag="ps1", name=f"ps1_{a}")
            nc.tensor.matmul(
                ps1[:, 0:128],
                lhsT=lhsT1(0, a),
                rhs=bt[:, GV0 : GV0 + 128],
                start=True,
                stop=False,
            )
            for w in (1, 2, 3):
                nc.tensor.matmul(
                    ps1[:, 128 * (w - 1) : 128 * (w + 1)],
                    lhsT=lhsT1(w, a),
                    rhs=bt[:, GV1 : GV1 + 256],
                    start=False,
                    stop=False,
                )
            nc.tensor.matmul(
                ps1[:, 384:512],
                lhsT=lhsT1(4, a),
                rhs=bt[:, GV1 : GV1 + 128],
                start=False,
                stop=True,
            )
            cast(tm[:, 512 * a : 512 * (a + 1)], ps1[:])

        def tmv(b, q):
            return tm[:, 512 * b + 128 * q : 512 * b + 128 * (q + 1)]

        def pass2_tile(g, q, st, stoff):
            """One 128-row block q of output cols [512*g, +512)."""
            ps2 = p2_pool.tile([128, 512], DT, tag="ps2", name=f"ps2_{g}_{q}")
            b0 = 4 * g
            nc.tensor.matmul(
                ps2[:, 0:128],
                lhsT=tmv(b0, q),
                rhs=bt[:, GH0 : GH0 + 128],
                start=True,
                stop=False,
            )
            for bl in (1, 2, 3):
                nc.tensor.matmul(
                    ps2[:, 128 * (bl - 1) : 128 * (bl + 1)],
                    lhsT=tmv(b0 + bl, q),
                    rhs=bt[:, GH1 : GH1 + 256],
                    start=False,
                    stop=False,
                )
            nc.tensor.matmul(
                ps2[:, 384:512],
                lhsT=tmv(b0 + 4, q),
                rhs=bt[:, GH1 : GH1 + 128],
                start=False,
                stop=True,
            )
            cast(st[:, stoff(q) : stoff(q) + 512], ps2[:])
            if g == 7:
                # final group: per-q 512-col DMA — minimal bytes behind the
                # last matmul (group 6 already flushed separately)
                dma(
                    y[128 * q : 128 * (q + 1), 3584:4096],
                    st[:, 512 * q : 512 * (q + 1)],
                )
            elif g == 6 and q == 3:
                # group 6 flushes as soon as its casts land, off the tail
                yv = y.rearrange("(q p) c -> p q c", q=4)[:, :, 3072:3584]
                sv = st.rearrange("p (q c) -> p q c", q=4)
                dma(yv, sv)
            elif g % 2 == 1 and q == 3:
                # pair-of-groups DMA: 1024-col spans make 2KB per-partition
                # runs, halving the output packet count (engine-rate bound)
                gp = g // 2
                yv = y.rearrange("(q p) c -> p q c", q=4)[
                    :, :, 1024 * gp : 1024 * (gp + 1)
                ]
                sv = st.rearrange("p (q c) -> p q c", q=4)
                dma(yv, sv)

        # pass1 a-units with pass2 tiles interleaved: group g needs tm
        # windows up to 4*g+4; its four q-tiles drain one per a-unit so
        # PSUM bank reuse never stalls the PE. Groups 0-5 stage output in
        # pair tiles (gp = g//2); groups 6/7 get their own finer tiles.
        pending = []
        sts = {}
        for a in range(NA):
            pass1_a(a)
            if a >= 4 and a % 4 == 0:
                g = a // 4 - 1
                if g >= 6:
                    sts[g] = st_pool.tile(
                        [128, 2048], mm_dtype, name=f"stf_{g}", tag="stf"
                    )
                elif g % 2 == 0:
                    gp = g // 2
                    sts[g] = sts[g + 1] = st_pool.tile(
                        [128, 4096], mm_dtype, name=f"stp_{gp}", tag="stp"
                    )
                pending += [(g, q) for q in range(4)]
            if pending:
                g, q = pending.pop(0)
                pass2_tile(g, q, sts[g], _stoff(g))
        for g, q in pending:
            pass2_tile(g, q, sts[g], _stoff(g))

    nc.compile()
    return nc


def _get_nc(mm_dtype):
    key = str(mm_dtype)
    if key not in _compiled:
        _compiled[key] = _build_nc(mm_dtype)
    return _compiled[key]


def _make_band(g, d):
    # G_d[r, c] = g[r - c + 128*d], zero outside [0, TAPS)
    idx = np.arange(128)[:, None] - np.arange(128)[None, :] + 128 * d
    valid = (idx >= 0) & (idx < TAPS)
    return np.where(valid, g[np.clip(idx, 0, TAPS - 1)], 0.0).astype(np.float32)


def kernel(x: np.ndarray, weight: np.ndarray) -> np.ndarray:
    x = np.asarray(x, dtype=np.float32)
    Wm = np.asarray(weight, dtype=np.float32).reshape(TAPS, TAPS)
    assert x.shape == (H, W), x.shape

    # rank-1 (separable) decomposition of the 2D kernel
    u, s, vt = np.linalg.svd(Wm.astype(np.float64))
    gv = (u[:, 0] * np.sqrt(s[0]))
    gh = (vt[0] * np.sqrt(s[0]))
    if gv.sum() < 0:
        gv, gh = -gv, -gh
    gv = gv.astype(np.float32)
    gh = gh.astype(np.float32)

    np_dt = np.float16
    bandst = np.concatenate(
        [_make_band(gv, 1), _make_band(gv, 0), _make_band(gh, 1), _make_band(gh, 0)],
        axis=1,
    ).astype(np_dt)

    # padded fp16 plane; strip for core c is rows [c*RPC, c*RPC + 640),
    # then relaid chunk-major: chunk ci holds its 5 row windows side by side
    xpad = np.zeros((H + 128, XP_COLS), np_dt)
    xpad[PAD : PAD + H, PAD : PAD + W] = x.astype(np_dt)
    in_maps = []
    for c in range(N_CORES):
        r0 = c * RPC
        strip = xpad[r0 : r0 + XP_ROWS]
        xp = np.empty((128, NW1 * XP_COLS), np_dt)
        for ci in range(len(CCUTS) - 1):
            cs, ce = CCUTS[ci], CCUTS[ci + 1]
            cw = ce - cs
            for w in range(NW1):
                off = 5 * cs + w * cw
                xp[:, off : off + cw] = strip[128 * w : 128 * (w + 1), cs:ce]
        in_maps.append({"xp": xp, "bandst": bandst})

    mm_dtype = mybir.dt.float16
    nc = _get_nc(mm_dtype)

    trace = os.environ.get("BLUR_TRACE") == "1"
    res = None
    last_exc = None
    for attempt in range(3):
        try:
            res = bass_utils.run_bass_kernel_spmd(
                nc, in_maps, core_ids=list(range(N_CORES)), trace=trace
            )
            break
        except Exception as e:  # transient NRT/device blips — retry
            last_exc = e
            time.sleep(2.0)
    if res is None:
        raise last_exc
    if trace:
        print(f"HW exec time: {res.exec_time_ns} ns")
        print(f"mean exec time: {res.mean_exec_time_ns} ns")
        if res.instructions_and_trace is not None:
            print(f"trace: {res.instructions_and_trace[1]}")

    out = np.concatenate(
        [res.results[c]["y"].astype(np.float32) for c in range(N_CORES)], axis=0
    )
    return out[None, None]
